# revision 8
# baseline (speedup 1.0000x reference)
"""Trainium2 Bass kernel for nn_DFTQNN_81776177316168.

reference: probs = |U_24 ... U_1 psi|^2 with U_k = expm(-i theta_k G_k),
G_k Hermitian 1024x1024 (symmetrized complex gaussian), psi = normalized
padded feature.

Strategy (expert-parallel on the gate axis, 3 gates per core):
  - Only U_k @ psi is ever needed, so the device never forms
    expm(-i theta G) itself. Per gate it computes a degree-3 Chebyshev
    polynomial V ~ exp(-iM) of the scaled generator M = (theta/2^s) G
    (spectrum in [-X0, X0]); the host then applies V to psi 2^s times
    in float64 (the scaling-and-squaring steps become cheap matvecs).
  - The polynomial is evaluated in Horner form so both device matmuls
    use host-provided Hermitian operands as the stationary side:
        W = M @ B1   (+ c1 I fused into the eviction)
        V = M @ W    (+ c0 I fused), with B1 = c2 I + c3 M from host.
  - Complex products use Gauss's 3-multiplication trick:
        P1 = Mr X_r, P2 = Mi X_i, P3 = (Mr+Mi)(X_r+X_i)
        O_re = P1 - P2, O_im = P3 - P1 - P2
    The sum operands come free: host precomputes B1r+B1i (and the
    Hermitian lhsT planes mr/mn/ms need no transposes); the W eviction
    writes Wr+Wi as a third pair plane.
  - Each real product runs as fp16 hi/lo split pairs (Dekker):
    X = X_h + X_l/2048, both fp16, side by side in one [1024, 2048]
    "pair plane". A product A*B = A_h B_h + (A_h B_l + A_l B_h)/2048
    accumulates main and cross terms in separate PSUM banks (fp32),
    combined on the DVE at eviction (~2^-22 relative error). Matmuls
    are emitted with consecutive instructions sharing the stationary
    operand; a post-compile pass deletes the redundant LDWEIGHTS that
    legalization inserts (1 load per 1.5 matmuls instead of 1:1).
"""

import math
from contextlib import ExitStack

import numpy as np

D = 1024           # statevector dim
P = 128            # partitions
NB = D // P        # 8 row blocks
CB = 512           # matmul moving free dim = one fp32 PSUM bank
NCOL = D // CB     # 2 col blocks
NK = 24            # gates
NCORES = 8
GPC = NK // NCORES # gates (slots) per core
LAM_BOUND = 64.3 * 1.06   # GUE edge 2*sqrt(D) with margin
X0 = 0.1           # max scaled spectral radius after 2^-s scaling
LOSC = 2048.0      # lo-plane scale (2^11)

_prog_cache = {}

# test-harness hooks: when TRACE is set, the SPMD run captures an NTFF
# profile and the BassKernelResults lands in LAST_RESULT.
TRACE = False
LAST_RESULT = None

IN_NAMES = ("mr", "mn", "ms", "b1r", "b1i", "b1s")  # pair planes [D, 2D]


def _cheb_coeffs(x0, deg):
    """Power-basis coeffs of the Chebyshev interpolant of exp(-ix) on
    [-x0, x0]."""
    from numpy.polynomial import chebyshev as Cb
    n = deg + 1
    xk = np.cos(np.pi * (np.arange(n) + 0.5) / n)
    fv = np.exp(-1j * x0 * xk)
    Tm = np.cos(np.outer(np.arange(n), np.arccos(xk)))
    ck = 2.0 / n * (Tm @ fv)
    ck[0] *= 0.5
    p = Cb.cheb2poly(ck)
    return p * (1.0 / x0) ** np.arange(n)


def _build_program():
    import concourse.bacc as bacc
    import concourse.tile as tile
    import concourse.mybir as mybir

    dt = mybir.dt
    f32 = dt.float32
    f16 = dt.float16
    AL = mybir.AluOpType
    D2 = 2 * D

    nc = bacc.Bacc("TRN2", target_bir_lowering=False, debug=False,
                   num_devices=NCORES)

    m_in = [{nmm: nc.dram_tensor(f"{nmm}{j}", [D, D2], f16,
                                 kind="ExternalInput").ap()
             for nmm in IN_NAMES} for j in range(GPC)]
    dg_in = [nc.dram_tensor(f"dg{j}", [P, 4 * P], f32,
                            kind="ExternalInput").ap() for j in range(GPC)]
    u_out = [(nc.dram_tensor(f"u{j}re", [D, D], f32, kind="ExternalOutput").ap(),
              nc.dram_tensor(f"u{j}im", [D, D], f32, kind="ExternalOutput").ap())
             for j in range(GPC)]

    uid = [0]

    def nm(base):
        uid[0] += 1
        return f"{base}_{uid[0]}"

    with tile.TileContext(nc) as tc, ExitStack() as ctx:
        dram = ctx.enter_context(tc.tile_pool(name="dram", bufs=1,
                                              space="DRAM"))
        xst = ctx.enter_context(tc.tile_pool(name="xst", bufs=2))
        lst = ctx.enter_context(tc.tile_pool(name="lst", bufs=2))
        est = ctx.enter_context(tc.tile_pool(name="est", bufs=16))
        evh = ctx.enter_context(tc.tile_pool(name="evh", bufs=8))
        ps = ctx.enter_context(tc.tile_pool(name="ps", bufs=1, space="PSUM"))
        cst = ctx.enter_context(tc.tile_pool(name="cst", bufs=1))

        # per-slot diag coeff tiles: [c1re*I | c1im*I | c0re*I | c0im*I]
        dgt = []
        for j in range(GPC):
            t = cst.tile([P, 4 * P], f32, tag=f"dg{j}", name=nm("dgt"))
            nc.sync.dma_start(t[:], dg_in[j])
            dgt.append(t)

        # per-slot W pair planes in DRAM: re, im, and re+im (Gauss sum)
        wpl = [tuple(dram.tile([D, D2], f16, tag=f"w{j}{sfx}",
                               name=nm("w"))[:, :]
                     for sfx in ("r", "i", "s")) for j in range(GPC)]

        def stage_half(plane, n, tag):
            """rhs stage of one 512-col block: [P, NB*D] f16, per row
            block kb holding [hi CB | lo CB]."""
            t = xst.tile([P, NB * D], f16, tag=tag, name=nm(tag))
            for kb in range(NB):
                src = plane[kb * P:(kb + 1) * P, :].rearrange(
                    "q (h m) -> q h m", h=2)[:, :, n * CB:(n + 1) * CB]
                nc.sync.dma_start(
                    t[:, kb * D:(kb + 1) * D].rearrange(
                        "p (h c) -> p h c", h=2), src)
            return t

        def xslh(t, kb, half):
            base = kb * D + half * CB
            return t[:, base: base + CB]

        def stage_cols(plane, p0, tag):
            """lhsT col-block stage: [P, 2*NB*P]; half-major then kb."""
            t = lst.tile([P, 2 * NB * P], f16, tag=tag, name=nm(tag))
            for half in range(2):
                srcv = plane.rearrange("(kb q) m2 -> q kb m2", q=P)[
                    :, :, half * D + p0 * P: half * D + (p0 + 1) * P]
                nc.sync.dma_start(
                    t[:, half * NB * P:(half + 1) * NB * P].rearrange(
                        "p (kb m) -> p kb m", kb=NB), srcv)
            return t

        def lsl(t, kb, half):
            base = half * NB * P + kb * P
            return t[:, base: base + P]

        qctr = [0]

        def psum6():
            b0 = (6 * qctr[0]) % 8
            qctr[0] += 1
            return [ps.tile([P, CB], f32, tag=f"pb{(b0 + i) % 8}",
                            name=nm("pq")) for i in range(6)]

        def matmul_g(L3, X3, evict):
            """O = L^T @ X complex via Gauss 3-mult on fp16 Dekker pair
            planes. L3 = lhsT planes of (P1, P2, P3); X3 = (Xr, Xi, Xs)
            pair planes with Xs = Xr + Xi. evict(p0, n, banks) with
            banks = [P1m, P1c, P2m, P2c, P3m, P3c]."""
            for n in range(NCOL):
                xt3 = [stage_half(X3[i], n, tg)
                       for i, tg in enumerate(("xr", "xi", "xs"))]
                for p0 in range(NB):
                    lt3 = [stage_cols(L3[i], p0, tg)
                           for i, tg in enumerate(("lr", "ln", "ls"))]
                    banks = psum6()
                    for pi in range(3):
                        lt, xt = lt3[pi], xt3[pi]
                        Bm, Bc = banks[2 * pi], banks[2 * pi + 1]
                        for kb in range(NB):
                            # hi lhsT serves main + one cross matmul
                            nc.tensor.matmul(
                                Bm[:], lsl(lt, kb, 0), xslh(xt, kb, 0),
                                start=(kb == 0), stop=(kb == NB - 1),
                                skip_group_check=True)
                            nc.tensor.matmul(
                                Bc[:], lsl(lt, kb, 0), xslh(xt, kb, 1),
                                start=(kb == 0), stop=False,
                                skip_group_check=True)
                            nc.tensor.matmul(
                                Bc[:], lsl(lt, kb, 1), xslh(xt, kb, 0),
                                start=False, stop=(kb == NB - 1),
                                skip_group_check=True)
                    evict(p0, n, banks)

        def osl32(plane, p0, n):
            return plane[p0 * P:(p0 + 1) * P, n * CB:(n + 1) * CB]

        def pair_dst(plane, p0, n):
            return plane[p0 * P:(p0 + 1) * P, :].rearrange(
                "p (h c) -> p h c", h=2)[:, :, n * CB:(n + 1) * CB]

        def combine6(banks):
            """(O_re, O_im) fp32 from the 6 PSUM banks. Each DVE op reads
            at most one PSUM operand (single PSUM read port); ordered so
            banks release as early as possible for the next quad."""
            P1m, P1c, P2m, P2c, P3m, P3c = banks
            ti = lambda: est.tile([P, CB], f32, tag="ev", name=nm("cb"))
            sc = 1.0 / LOSC

            def stt(dst, a, s, b):
                nc.vector.scalar_tensor_tensor(dst[:], a[:], s, b[:],
                                               op0=AL.mult, op1=AL.add)
            ta = ti(); nc.vector.tensor_copy(ta[:], P1m[:])
            v0 = ti(); nc.vector.tensor_sub(v0[:], P3m[:], ta[:])
            u0 = ti(); nc.vector.tensor_sub(u0[:], ta[:], P2m[:])
            v1 = ti(); nc.vector.tensor_sub(v1[:], v0[:], P2m[:])
            u1 = ti(); stt(u1, P1c, sc, u0)
            v2 = ti(); stt(v2, P1c, -sc, v1)
            u2 = ti(); stt(u2, P2c, -sc, u1)
            v3 = ti(); stt(v3, P2c, -sc, v2)
            v4 = ti(); stt(v4, P3c, sc, v3)
            return u2, v4

        def diag_add(t, p0, n, dcol):
            if n == p0 // (CB // P):
                off = (p0 % (CB // P)) * P
                nc.vector.tensor_add(t[:, off:off + P], t[:, off:off + P],
                                     dcol)

        def split_out(t, plane, p0, n):
            """Pair-plane write of fp32 tile t; hi cast on the scalar
            engine, residue ops on gpsimd (DVE stays on PSUM work)."""
            hl = evh.tile([P, 2 * CB], f16, tag="evh", name=nm("hl"))
            nc.scalar.copy(hl[:, 0:CB], t[:])
            r = est.tile([P, CB], f32, tag="ev", name=nm("rr"))
            nc.gpsimd.tensor_sub(r[:], t[:], hl[:, 0:CB])
            nc.gpsimd.tensor_scalar_mul(hl[:, CB:2 * CB], r[:], LOSC)
            nc.sync.dma_start(pair_dst(plane, p0, n),
                              hl[:].rearrange("p (h c) -> p h c", h=2))

        def evict_W(j):
            def ev(p0, n, banks):
                u2, v4 = combine6(banks)
                diag_add(u2, p0, n, dgt[j][:, 0:P])
                diag_add(v4, p0, n, dgt[j][:, P:2 * P])
                ws = est.tile([P, CB], f32, tag="ev", name=nm("ws"))
                nc.vector.tensor_add(ws[:], u2[:], v4[:])
                split_out(u2, wpl[j][0], p0, n)
                split_out(v4, wpl[j][1], p0, n)
                split_out(ws, wpl[j][2], p0, n)
            return ev

        def evict_V(j):
            def ev(p0, n, banks):
                u2, v4 = combine6(banks)
                diag_add(u2, p0, n, dgt[j][:, 2 * P:3 * P])
                diag_add(v4, p0, n, dgt[j][:, 3 * P:4 * P])
                nc.sync.dma_start(osl32(u_out[j][0], p0, n), u2[:])
                nc.sync.dma_start(osl32(u_out[j][1], p0, n), v4[:])
            return ev

        # all W-matmuls first, then all V-matmuls: by the time slot j's
        # second matmul issues, its W finished two full matmuls ago, so
        # the PE never waits on an eviction->restage roundtrip.
        for j in range(GPC):
            mm = m_in[j]
            matmul_g((mm["mr"], mm["mn"], mm["ms"]),
                     (mm["b1r"], mm["b1i"], mm["b1s"]), evict_W(j))
        for j in range(GPC):
            mm = m_in[j]
            matmul_g((mm["mr"], mm["mn"], mm["ms"]), wpl[j], evict_V(j))

    nc.compile()
    _dedupe_ldweights(nc)
    return nc


def _dedupe_ldweights(nc):
    """Drop InstLdweights whose stationary operand is already loaded.

    Legalization inserts one LDWEIGHTS per matmul; consecutive matmuls
    sharing a stationary operand make ~1/3 of the loads redundant. The
    PE keeps loaded weights across (non-self-loading) matmuls, so a
    repeat load of the identical SBUF access pattern can be deleted
    once its sync deps are folded into the following matmul. Nothing
    references LDWEIGHTS by name (checked: zero inbound dependency
    edges), so deletion is safe."""
    ndrop = 0
    for f in nc.m.functions:
        for bb in f.blocks:
            insts = list(bb.instructions)
            loaded = None      # AP string currently in the PE array
            drop = set()
            pending = None     # deleted ld awaiting dep-merge into next mm
            for inst in insts:
                tn = type(inst).__name__
                if tn == "InstLdweights":
                    w = str(inst.ins[0])
                    if w == loaded:
                        drop.add(inst.name)
                        pending = inst
                    else:
                        loaded = w
                        pending = None
                elif tn == "InstMatmult":
                    if pending is not None:
                        inst.add_sync_dependencies_from(
                            pending.sync_dependency_set_copy())
                        inst.add_nosync_dependencies_from(
                            pending.nosync_dependency_set_copy())
                        pending = None
            if drop:
                ndrop += len(drop)
                bb.instructions = [x for x in insts if x.name not in drop]
    return ndrop


def _get_program():
    if "p" not in _prog_cache:
        _prog_cache["p"] = _build_program()
    return _prog_cache["p"]


def _split_pair(x32):
    h = x32.astype(np.float16)
    l = ((x32 - h.astype(np.float32)) * np.float32(LOSC)).astype(np.float16)
    return np.ascontiguousarray(np.concatenate([h, l], axis=1))


def kernel(feature, theta, gens_re, gens_im):
    feature = np.asarray(feature)
    th = np.asarray(theta)[:, 0].astype(np.float64)
    gens_re = np.asarray(gens_re)
    gens_im = np.asarray(gens_im)

    nc = _get_program()

    a = np.abs(th) * LAM_BOUND
    svals = [max(0, math.ceil(math.log2(max(float(a[k]), 1e-9) / X0)))
             for k in range(NK)]

    ident = np.eye(P, dtype=np.float32)
    in_maps = []
    for c in range(NCORES):
        m = {}
        for j in range(GPC):
            k = j * NCORES + c
            s = svals[k]
            cc = np.float32(0.5 * th[k] / (2.0 ** s))
            r = gens_re[k].astype(np.float32)
            im = gens_im[k].astype(np.float32)
            Mr = cc * (r + r.T)
            Mi = cc * (im - im.T)
            xeff = a[k] / (2.0 ** s)
            c0, c1, c2, c3 = _cheb_coeffs(xeff, 3)
            If = ident_full()
            B1r = (np.float32(c2.real) * If
                   + np.float32(c3.real) * Mr - np.float32(c3.imag) * Mi)
            B1i = (np.float32(c2.imag) * If
                   + np.float32(c3.imag) * Mr + np.float32(c3.real) * Mi)
            m[f"mr{j}"] = _split_pair(Mr)
            m[f"mn{j}"] = _split_pair(-Mi)
            m[f"ms{j}"] = _split_pair(Mr - Mi)
            m[f"b1r{j}"] = _split_pair(B1r)
            m[f"b1i{j}"] = _split_pair(B1i)
            m[f"b1s{j}"] = _split_pair(B1r + B1i)
            dg = np.zeros((P, 4 * P), np.float32)
            for col, v in enumerate((c1.real, c1.imag, c0.real, c0.imag)):
                dg[:, col * P:(col + 1) * P] = np.float32(v) * ident
            m[f"dg{j}"] = dg
        in_maps.append(m)

    from concourse.bass_utils import run_bass_kernel_spmd
    res = run_bass_kernel_spmd(nc, in_maps, core_ids=list(range(NCORES)),
                               trace=TRACE)
    global LAST_RESULT
    LAST_RESULT = res

    psi = np.zeros(D, np.complex128)
    psi[:feature.shape[0]] = feature.astype(np.float64)
    psi /= np.linalg.norm(psi)
    for k in range(NK):
        c, j = k % NCORES, k // NCORES
        V = (res.results[c][f"u{j}re"].astype(np.float64)
             + 1j * res.results[c][f"u{j}im"].astype(np.float64))
        for _ in range(2 ** svals[k]):
            psi = V @ psi
    return (np.abs(psi) ** 2).astype(np.float32)


_IDENT_FULL = None


def ident_full():
    global _IDENT_FULL
    if _IDENT_FULL is None:
        _IDENT_FULL = np.eye(D, dtype=np.float32)
    return _IDENT_FULL


# revision 9
# speedup vs baseline: 1.3565x; 1.3565x over previous
"""Trainium2 Bass kernel for nn_DFTQNN_81776177316168.

reference: probs = |U_24 ... U_1 psi|^2 with U_k = expm(-i theta_k G_k),
G_k Hermitian 1024x1024 (symmetrized complex gaussian), psi = normalized
padded feature.

Strategy (expert-parallel on the gate axis, 3 gates per core):
  - Only U_k @ psi is ever needed, so the device never forms
    expm(-i theta G) itself. Per gate it computes a degree-3 Chebyshev
    polynomial V ~ exp(-iM) of the scaled generator M = (theta/2^s) G
    (spectrum in [-X0, X0]); the host then applies V to psi 2^s times
    in float64 (the scaling-and-squaring steps become cheap matvecs).
  - The polynomial is evaluated in Horner form so both device matmuls
    use host-provided Hermitian operands as the stationary side:
        W = M @ B1   (+ c1 I fused into the eviction)
        V = M @ W    (+ c0 I fused), with B1 = c2 I + c3 M from host.
  - Complex products use Gauss's 3-multiplication trick:
        P1 = Mr X_r, P2 = Mi X_i, P3 = (Mr+Mi)(X_r+X_i)
        O_re = P1 - P2, O_im = P3 - P1 - P2
    The sum operands come free: host precomputes B1r+B1i (and the
    Hermitian lhsT planes mr/mn/ms need no transposes); the W eviction
    writes Wr+Wi as a third pair plane.
  - Each real product runs as fp16 hi/lo split pairs (Dekker):
    X = X_h + X_l/2048, both fp16, side by side in one [1024, 2048]
    "pair plane". A product A*B = A_h B_h + (A_h B_l + A_l B_h)/2048
    accumulates main and cross terms in separate PSUM banks (fp32),
    combined on the DVE at eviction (~2^-22 relative error). Matmuls
    are emitted with consecutive instructions sharing the stationary
    operand; a post-compile pass deletes the redundant LDWEIGHTS that
    legalization inserts (1 load per 1.5 matmuls instead of 1:1).
"""

import math
from contextlib import ExitStack

import numpy as np

D = 1024           # statevector dim
P = 128            # partitions
NB = D // P        # 8 row blocks
CB = 512           # matmul moving free dim = one fp32 PSUM bank
NCOL = D // CB     # 2 col blocks
NK = 24            # gates
NCORES = 8
GPC = NK // NCORES # gates (slots) per core
LAM_BOUND = 64.3 * 1.06   # GUE edge 2*sqrt(D) with margin
X0 = 0.1           # max scaled spectral radius after 2^-s scaling
LOSC = 2048.0      # lo-plane scale (2^11)

_prog_cache = {}

# test-harness hooks: when TRACE is set, the SPMD run captures an NTFF
# profile and the BassKernelResults lands in LAST_RESULT.
TRACE = False
LAST_RESULT = None

IN_NAMES = ("mr", "mn", "ms", "b1r", "b1i", "b1s")  # pair planes [D, 2D]


def _cheb_coeffs(x0, deg):
    """Power-basis coeffs of the Chebyshev interpolant of exp(-ix) on
    [-x0, x0]."""
    from numpy.polynomial import chebyshev as Cb
    n = deg + 1
    xk = np.cos(np.pi * (np.arange(n) + 0.5) / n)
    fv = np.exp(-1j * x0 * xk)
    Tm = np.cos(np.outer(np.arange(n), np.arccos(xk)))
    ck = 2.0 / n * (Tm @ fv)
    ck[0] *= 0.5
    p = Cb.cheb2poly(ck)
    return p * (1.0 / x0) ** np.arange(n)


def _build_program():
    import concourse.bacc as bacc
    import concourse.tile as tile
    import concourse.mybir as mybir

    dt = mybir.dt
    f32 = dt.float32
    f16 = dt.float16
    AL = mybir.AluOpType
    D2 = 2 * D

    nc = bacc.Bacc("TRN2", target_bir_lowering=False, debug=False,
                   num_devices=NCORES)

    m_in = [{nmm: nc.dram_tensor(f"{nmm}{j}", [D, D2], f16,
                                 kind="ExternalInput").ap()
             for nmm in IN_NAMES} for j in range(GPC)]
    dg_in = [nc.dram_tensor(f"dg{j}", [P, 4 * P], f32,
                            kind="ExternalInput").ap() for j in range(GPC)]
    u_out = [(nc.dram_tensor(f"u{j}re", [D, D], f32, kind="ExternalOutput").ap(),
              nc.dram_tensor(f"u{j}im", [D, D], f32, kind="ExternalOutput").ap())
             for j in range(GPC)]

    uid = [0]

    def nm(base):
        uid[0] += 1
        return f"{base}_{uid[0]}"

    with tile.TileContext(nc) as tc, ExitStack() as ctx:
        dram = ctx.enter_context(tc.tile_pool(name="dram", bufs=1,
                                              space="DRAM"))
        xst = ctx.enter_context(tc.tile_pool(name="xst", bufs=2))
        lst = ctx.enter_context(tc.tile_pool(name="lst", bufs=2))
        est = ctx.enter_context(tc.tile_pool(name="est", bufs=16))
        evh = ctx.enter_context(tc.tile_pool(name="evh", bufs=8))
        ps = ctx.enter_context(tc.tile_pool(name="ps", bufs=1, space="PSUM"))
        cst = ctx.enter_context(tc.tile_pool(name="cst", bufs=1))

        # per-slot diag coeff tiles: [c1re*I | c1im*I | c0re*I | c0im*I]
        dgt = []
        for j in range(GPC):
            t = cst.tile([P, 4 * P], f32, tag=f"dg{j}", name=nm("dgt"))
            nc.sync.dma_start(t[:], dg_in[j])
            dgt.append(t)

        # per-slot W pair planes in DRAM: re, im, and re+im (Gauss sum)
        wpl = [tuple(dram.tile([D, D2], f16, tag=f"w{j}{sfx}",
                               name=nm("w"))[:, :]
                     for sfx in ("r", "i", "s")) for j in range(GPC)]

        def stage_half(plane, n, tag):
            """rhs stage of one 512-col block: [P, NB*D] f16, per row
            block kb holding [hi CB | lo CB]."""
            t = xst.tile([P, NB * D], f16, tag=tag, name=nm(tag))
            for kb in range(NB):
                src = plane[kb * P:(kb + 1) * P, :].rearrange(
                    "q (h m) -> q h m", h=2)[:, :, n * CB:(n + 1) * CB]
                nc.sync.dma_start(
                    t[:, kb * D:(kb + 1) * D].rearrange(
                        "p (h c) -> p h c", h=2), src)
            return t

        def xslh(t, kb, half):
            base = kb * D + half * CB
            return t[:, base: base + CB]

        def stage_cols(plane, p0, tag):
            """lhsT col-block stage: [P, 2*NB*P]; half-major then kb."""
            t = lst.tile([P, 2 * NB * P], f16, tag=tag, name=nm(tag))
            for half in range(2):
                srcv = plane.rearrange("(kb q) m2 -> q kb m2", q=P)[
                    :, :, half * D + p0 * P: half * D + (p0 + 1) * P]
                nc.sync.dma_start(
                    t[:, half * NB * P:(half + 1) * NB * P].rearrange(
                        "p (kb m) -> p kb m", kb=NB), srcv)
            return t

        def lsl(t, kb, half):
            base = half * NB * P + kb * P
            return t[:, base: base + P]

        qctr = [0]

        def psum6():
            b0 = (6 * qctr[0]) % 8
            qctr[0] += 1
            return [ps.tile([P, CB], f32, tag=f"pb{(b0 + i) % 8}",
                            name=nm("pq")) for i in range(6)]

        def matmul_g(L3, X3, evict):
            """O = L^T @ X complex via Gauss 3-mult on fp16 Dekker pair
            planes. L3 = lhsT planes of (P1, P2, P3); X3 = (Xr, Xi, Xs)
            pair planes with Xs = Xr + Xi. evict(p0, n, banks) with
            banks = [P1m, P1c, P2m, P2c, P3m, P3c]."""
            for n in range(NCOL):
                xt3 = [stage_half(X3[i], n, tg)
                       for i, tg in enumerate(("xr", "xi", "xs"))]
                for p0 in range(NB):
                    lt3 = [stage_cols(L3[i], p0, tg)
                           for i, tg in enumerate(("lr", "ln", "ls"))]
                    banks = psum6()
                    for pi in range(3):
                        lt, xt = lt3[pi], xt3[pi]
                        Bm, Bc = banks[2 * pi], banks[2 * pi + 1]
                        for kb in range(NB):
                            # hi lhsT serves main + one cross matmul
                            nc.tensor.matmul(
                                Bm[:], lsl(lt, kb, 0), xslh(xt, kb, 0),
                                start=(kb == 0), stop=(kb == NB - 1),
                                skip_group_check=True)
                            nc.tensor.matmul(
                                Bc[:], lsl(lt, kb, 0), xslh(xt, kb, 1),
                                start=(kb == 0), stop=False,
                                skip_group_check=True)
                            nc.tensor.matmul(
                                Bc[:], lsl(lt, kb, 1), xslh(xt, kb, 0),
                                start=False, stop=(kb == NB - 1),
                                skip_group_check=True)
                    evict(p0, n, banks)

        def osl32(plane, p0, n):
            return plane[p0 * P:(p0 + 1) * P, n * CB:(n + 1) * CB]

        def pair_dst(plane, p0, n):
            return plane[p0 * P:(p0 + 1) * P, :].rearrange(
                "p (h c) -> p h c", h=2)[:, :, n * CB:(n + 1) * CB]

        def combine6(banks):
            """(O_re, O_im) fp32 from the 6 PSUM banks. Each DVE op reads
            at most one PSUM operand (single PSUM read port); ordered so
            banks release as early as possible for the next quad."""
            P1m, P1c, P2m, P2c, P3m, P3c = banks
            ti = lambda: est.tile([P, CB], f32, tag="ev", name=nm("cb"))
            sc = 1.0 / LOSC

            def stt(dst, a, s, b):
                nc.vector.scalar_tensor_tensor(dst[:], a[:], s, b[:],
                                               op0=AL.mult, op1=AL.add)
            ta = ti(); nc.vector.tensor_copy(ta[:], P1m[:])
            v0 = ti(); nc.vector.tensor_sub(v0[:], P3m[:], ta[:])
            u0 = ti(); nc.vector.tensor_sub(u0[:], ta[:], P2m[:])
            v1 = ti(); nc.vector.tensor_sub(v1[:], v0[:], P2m[:])
            u1 = ti(); stt(u1, P1c, sc, u0)
            v2 = ti(); stt(v2, P1c, -sc, v1)
            u2 = ti(); stt(u2, P2c, -sc, u1)
            v3 = ti(); stt(v3, P2c, -sc, v2)
            v4 = ti(); stt(v4, P3c, sc, v3)
            return u2, v4

        def diag_add(t, p0, n, dcol):
            if n == p0 // (CB // P):
                off = (p0 % (CB // P)) * P
                nc.vector.tensor_add(t[:, off:off + P], t[:, off:off + P],
                                     dcol)

        def split_out(t, plane, p0, n):
            """Pair-plane write of fp32 tile t; hi cast on the scalar
            engine, residue ops on gpsimd (DVE stays on PSUM work)."""
            hl = evh.tile([P, 2 * CB], f16, tag="evh", name=nm("hl"))
            nc.scalar.copy(hl[:, 0:CB], t[:])
            r = est.tile([P, CB], f32, tag="ev", name=nm("rr"))
            nc.gpsimd.tensor_sub(r[:], t[:], hl[:, 0:CB])
            nc.vector.tensor_scalar_mul(hl[:, CB:2 * CB], r[:], LOSC)
            nc.sync.dma_start(pair_dst(plane, p0, n),
                              hl[:].rearrange("p (h c) -> p h c", h=2))

        def evict_W(j):
            def ev(p0, n, banks):
                u2, v4 = combine6(banks)
                diag_add(u2, p0, n, dgt[j][:, 0:P])
                diag_add(v4, p0, n, dgt[j][:, P:2 * P])
                ws = est.tile([P, CB], f32, tag="ev", name=nm("ws"))
                nc.vector.tensor_add(ws[:], u2[:], v4[:])
                split_out(u2, wpl[j][0], p0, n)
                split_out(v4, wpl[j][1], p0, n)
                split_out(ws, wpl[j][2], p0, n)
            return ev

        def evict_V(j):
            def ev(p0, n, banks):
                u2, v4 = combine6(banks)
                diag_add(u2, p0, n, dgt[j][:, 2 * P:3 * P])
                diag_add(v4, p0, n, dgt[j][:, 3 * P:4 * P])
                nc.sync.dma_start(osl32(u_out[j][0], p0, n), u2[:])
                nc.sync.dma_start(osl32(u_out[j][1], p0, n), v4[:])
            return ev

        # all W-matmuls first, then all V-matmuls: by the time slot j's
        # second matmul issues, its W finished two full matmuls ago, so
        # the PE never waits on an eviction->restage roundtrip.
        for j in range(GPC):
            mm = m_in[j]
            matmul_g((mm["mr"], mm["mn"], mm["ms"]),
                     (mm["b1r"], mm["b1i"], mm["b1s"]), evict_W(j))
        for j in range(GPC):
            mm = m_in[j]
            matmul_g((mm["mr"], mm["mn"], mm["ms"]), wpl[j], evict_V(j))

    nc.compile()
    _dedupe_ldweights(nc)
    return nc


def _dedupe_ldweights(nc):
    """Drop InstLdweights whose stationary operand is already loaded.

    Legalization inserts one LDWEIGHTS per matmul; consecutive matmuls
    sharing a stationary operand make ~1/3 of the loads redundant. The
    PE keeps loaded weights across (non-self-loading) matmuls, so a
    repeat load of the identical SBUF access pattern can be deleted
    once its sync deps are folded into the following matmul. Nothing
    references LDWEIGHTS by name (checked: zero inbound dependency
    edges), so deletion is safe."""
    ndrop = 0
    for f in nc.m.functions:
        for bb in f.blocks:
            insts = list(bb.instructions)
            loaded = None      # AP string currently in the PE array
            drop = set()
            pending = None     # deleted ld awaiting dep-merge into next mm
            for inst in insts:
                tn = type(inst).__name__
                if tn == "InstLdweights":
                    w = str(inst.ins[0])
                    if w == loaded:
                        drop.add(inst.name)
                        pending = inst
                    else:
                        loaded = w
                        pending = None
                elif tn == "InstMatmult":
                    if pending is not None:
                        inst.add_sync_dependencies_from(
                            pending.sync_dependency_set_copy())
                        inst.add_nosync_dependencies_from(
                            pending.nosync_dependency_set_copy())
                        pending = None
            if drop:
                ndrop += len(drop)
                bb.instructions = [x for x in insts if x.name not in drop]
    return ndrop


def _get_program():
    if "p" not in _prog_cache:
        _prog_cache["p"] = _build_program()
    return _prog_cache["p"]


def _split_pair(x32):
    h = x32.astype(np.float16)
    l = ((x32 - h.astype(np.float32)) * np.float32(LOSC)).astype(np.float16)
    return np.ascontiguousarray(np.concatenate([h, l], axis=1))


def kernel(feature, theta, gens_re, gens_im):
    feature = np.asarray(feature)
    th = np.asarray(theta)[:, 0].astype(np.float64)
    gens_re = np.asarray(gens_re)
    gens_im = np.asarray(gens_im)

    nc = _get_program()

    a = np.abs(th) * LAM_BOUND
    svals = [max(0, math.ceil(math.log2(max(float(a[k]), 1e-9) / X0)))
             for k in range(NK)]

    ident = np.eye(P, dtype=np.float32)
    in_maps = []
    for c in range(NCORES):
        m = {}
        for j in range(GPC):
            k = j * NCORES + c
            s = svals[k]
            cc = np.float32(0.5 * th[k] / (2.0 ** s))
            r = gens_re[k].astype(np.float32)
            im = gens_im[k].astype(np.float32)
            Mr = cc * (r + r.T)
            Mi = cc * (im - im.T)
            xeff = a[k] / (2.0 ** s)
            c0, c1, c2, c3 = _cheb_coeffs(xeff, 3)
            If = ident_full()
            B1r = (np.float32(c2.real) * If
                   + np.float32(c3.real) * Mr - np.float32(c3.imag) * Mi)
            B1i = (np.float32(c2.imag) * If
                   + np.float32(c3.imag) * Mr + np.float32(c3.real) * Mi)
            m[f"mr{j}"] = _split_pair(Mr)
            m[f"mn{j}"] = _split_pair(-Mi)
            m[f"ms{j}"] = _split_pair(Mr - Mi)
            m[f"b1r{j}"] = _split_pair(B1r)
            m[f"b1i{j}"] = _split_pair(B1i)
            m[f"b1s{j}"] = _split_pair(B1r + B1i)
            dg = np.zeros((P, 4 * P), np.float32)
            for col, v in enumerate((c1.real, c1.imag, c0.real, c0.imag)):
                dg[:, col * P:(col + 1) * P] = np.float32(v) * ident
            m[f"dg{j}"] = dg
        in_maps.append(m)

    from concourse.bass_utils import run_bass_kernel_spmd
    res = run_bass_kernel_spmd(nc, in_maps, core_ids=list(range(NCORES)),
                               trace=TRACE)
    global LAST_RESULT
    LAST_RESULT = res

    psi = np.zeros(D, np.complex128)
    psi[:feature.shape[0]] = feature.astype(np.float64)
    psi /= np.linalg.norm(psi)
    for k in range(NK):
        c, j = k % NCORES, k // NCORES
        V = (res.results[c][f"u{j}re"].astype(np.float64)
             + 1j * res.results[c][f"u{j}im"].astype(np.float64))
        for _ in range(2 ** svals[k]):
            psi = V @ psi
    return (np.abs(psi) ** 2).astype(np.float32)


_IDENT_FULL = None


def ident_full():
    global _IDENT_FULL
    if _IDENT_FULL is None:
        _IDENT_FULL = np.eye(D, dtype=np.float32)
    return _IDENT_FULL
